# revision 2
# baseline (speedup 1.0000x reference)
"""Trainium2 Bass kernel for nn_ClassificationModel (CNN window encoder +
4-layer post-norm transformer + mean-pool classifier head).

Data parallel across 8 NeuronCores: batch N=64 -> 8 samples/core.

v2 rewrite vs v1:
  - QK generation batched over tokens (N=512) instead of per-sample N=128.
  - Scores computed pre-transposed (S^T = K^T-major matmul) so exp(S^T) is
    directly the AV lhsT: removes all per-head PE transposes + DVE copies.
  - No softmax max-subtraction (scores are O(1); exp is safe); softmax
    denominator comes free from an N=1 ones-column matmul per head.
  - temp folded into Wq/bq; bv folded through Wo; LN affine (g, be) folded
    into the transpose psum->sbuf copies (ACT scale/bias) and K=1 bias-row
    matmuls; classifier weights pre-folded with the last LN affine.
  - CNN pooling shift-DMAs batched (one big DMA per conv stage) and pooled
    tensors repacked to 128-partition tiles (fewer conv matmuls).
  - All weights packed into a handful of DRAM blobs (6 inputs total).
"""

import math
import sys

sys.path.insert(0, "/opt/trn_rl_repo")

import numpy as np
import ml_dtypes

import concourse.bass as bass
import concourse.mybir as mybir
import concourse.tile as tile
from concourse import bacc
from concourse.bass import AP
from concourse.bass_utils import run_bass_kernel_spmd

BF = ml_dtypes.bfloat16
F32 = mybir.dt.float32
BF16 = mybir.dt.bfloat16
AX = mybir.AxisListType
OP = mybir.AluOpType
AF = mybir.ActivationFunctionType

# model dims
N, L, W = 64, 128, 256
D, H, NL, DFF = 384, 8, 4, 1536
E = D // H  # 48
CH = [1, 4, 16, 64]
K = 7
NCORES = 8
RPC = N // NCORES          # samples per core = 8
R = RPC * L                # rows per core = 1024
TEMP = 1.0 / math.sqrt(E)
EPS = 1e-5

# conv block sizes (output positions per Toeplitz block)
B0, B1, B2 = 32, 8, 2
NB0, NB1, NB2 = 256 // B0, 128 // B1, 64 // B2  # 8, 16, 32


# ---------------------------------------------------------------------------
# conv source-block overlap enumeration (shared host/device)
# ---------------------------------------------------------------------------

# (Bout, src_size, nsrc, nch): source tiles are [src_size*nch(<=128), 128rows]
# conv0 reads xT tiles (128 pos, 1 ch); conv1 reads pooled0 tiles (32 pos,
# 4 ch); conv2 reads pooled1 tiles (8 pos, 16 ch)
CONV_GEOM = {
    0: (B0, 128, 2, 1),
    1: (B1, 32, 4, 4),
    2: (B2, 8, 8, 16),
}


def overlaps(conv, b):
    """source tiles overlapping output block b's input window; (src, delta)."""
    Bout, src_size, nsrc, _ = CONV_GEOM[conv]
    w0, w1 = Bout * b - 3, Bout * b + Bout + 3
    res = []
    for s in range(nsrc):
        lo, hi = s * src_size, (s + 1) * src_size
        if max(w0, lo) < min(w1, hi):
            res.append((s, lo - Bout * b))
    return res


def conv_deltas(conv):
    nb = {0: NB0, 1: NB1, 2: NB2}[conv]
    ds = sorted({d for b in range(nb) for _, d in overlaps(conv, b)})
    return ds


def _m_layout(conv, h, co):
    if conv == 0:
        return (h & 1) * 64 + (h >> 1) * 4 + co
    if conv == 1:
        return (h & 1) * 64 + (h >> 1) * 16 + co
    return h * 64 + co


def _toeplitz(conv, w):
    """w: (C_out, C_in, K). returns (nvar, src_size*nch, 128) f32."""
    Bout, src_size, _, nch = CONV_GEOM[conv]
    cout = w.shape[0]
    ds = conv_deltas(conv)
    T = np.zeros((len(ds), src_size * nch, 128), np.float32)
    for vi, delta in enumerate(ds):
        for hp in range(src_size):
            for h in range(Bout):
                k = delta + hp - h + 3
                if 0 <= k < K:
                    for co in range(cout):
                        for ci in range(nch):
                            T[vi, hp * nch + ci, _m_layout(conv, h, co)] = w[co, ci, k]
    return T


def _pe_np(l, d):
    pos = np.arange(l)[:, None].astype(np.float32)
    i = np.arange(d // 2)[None, :].astype(np.float32)
    ang = pos / np.power(10000.0, 2.0 * i / d)
    pe = np.zeros((l, d), np.float32)
    pe[:, 0::2] = np.sin(ang)
    pe[:, 1::2] = np.cos(ang)
    return pe


# ---------------------------------------------------------------------------
# host-side weight preparation: pack everything into a few blobs
# ---------------------------------------------------------------------------

NV0 = len(conv_deltas(0))
NV1 = len(conv_deltas(1))
NV2 = len(conv_deltas(2))

# bf16 const blob column layout
_CB_T0 = 0
_CB_T1 = _CB_T0 + NV0 * 128
_CB_T2 = _CB_T1 + NV1 * 128
_CB_WE = _CB_T2 + NV2 * 128
_CB_IDN = _CB_WE + 16 * D
_CB_ONE = _CB_IDN + 128          # ones col [128,1]
_CB_ROW1 = _CB_ONE + 1           # ones row at partition 0 [1,128]
_CB_EBR = _CB_ROW1 + 128         # embed bias row at partition 0 [1,384]
_CB_END = _CB_EBR + D

# f32 const blob column layout
_CF_B0E = 0
_CF_B1E = 1
_CF_B2E = 2
_CF_PE = 3                       # pe [128, 384]
_CF_IDN = _CF_PE + D             # f32 identity [128,128]
_CF_ONESL = _CF_IDN + 128        # 1/L col
_CF_CLSW = _CF_ONESL + 1         # cls w eff [128, 3]
_CF_EPS = _CF_CLSW + 3
_CF_CLSB = _CF_EPS + 1           # cls bias at partition 0 [1,1]
_CF_END = _CF_CLSB + 1

# per-layer bf16 weight blob "LW": [NL, 3, 128, 3072] (wq/wk unpadded; the
# per-head 48-col slices are used directly as M=48 lhsT)
_LW_WQ = 0
_LW_WK = 384
_LW_WV = 768
_LW_WO = 1152
_LW_W1 = 1536
_LW_END = 3072

# per-layer bf16 blob "W2": [NL, 128, 4608] (12 chunks x 384)
_W2_END = 4608

# per-layer bf16 blob "SM": [NL, 128, 1536]
_SM_GIN = 0                      # g_in bcast [128, 384]
_SM_G1 = 384                     # g1 bcast [128, 384]
_SM_BOR = 768                    # bo_row at partition 0 [1, 384]
_SM_B2R = 1152                   # b2_row at partition 0 [1, 384]
_SM_END = 1536

# per-layer f32 misc blob "MS": [NL, 128, 40]
_MS_BQ = 0                       # [128, 8]
_MS_BK = 8
_MS_GINQ = 16                    # per-chunk scale cols [128, 3]
_MS_BEINQ = 19
_MS_G1Q = 22
_MS_BE1Q = 25
_MS_B1R = 28                     # [128, 12]
_MS_END = 40


def host_prep(inp):
    f32 = np.float32
    g1 = np.asarray(inp["g1"], f32); be1 = np.asarray(inp["be1"], f32)
    g2 = np.asarray(inp["g2"], f32); be2 = np.asarray(inp["be2"], f32)
    Wo = np.asarray(inp["Wo"], f32)
    bv = np.asarray(inp["bv"], f32)

    # ---- bf16 const blob
    cb = np.zeros((128, _CB_END), f32)
    for c, (dst, wname) in enumerate((( _CB_T0, "conv_w0"), (_CB_T1, "conv_w1"),
                                      (_CB_T2, "conv_w2"))):
        T = _toeplitz(c, np.asarray(inp[wname], f32))
        for vi in range(T.shape[0]):
            cb[:T.shape[1], dst + vi * 128: dst + (vi + 1) * 128] = T[vi]
    ew = np.asarray(inp["embed_w"], f32)  # (2048, 384)
    We_r = np.zeros((16, 128, D), f32)
    idx = np.arange(128)
    for c in range(16):
        We_r[c] = ew[(idx % 64) * 32 + 2 * c + idx // 64]
    cb[:, _CB_WE:_CB_WE + 16 * D] = We_r.transpose(1, 0, 2).reshape(128, 16 * D)
    cb[:, _CB_IDN:_CB_IDN + 128] = np.eye(128, dtype=f32)
    cb[:, _CB_ONE] = 1.0
    cb[0, _CB_ROW1:_CB_ROW1 + 128] = 1.0
    cb[0, _CB_EBR:_CB_EBR + D] = np.asarray(inp["embed_b"], f32)

    # ---- f32 const blob
    cf = np.zeros((128, _CF_END), f32)
    p = np.arange(128)
    cf[:, _CF_B0E] = np.asarray(inp["conv_b0"], f32)[p % 4]
    cf[:, _CF_B1E] = np.asarray(inp["conv_b1"], f32)[p % 16]
    cf[:, _CF_B2E] = np.asarray(inp["conv_b2"], f32)[p % 64]
    cf[:, _CF_PE:_CF_PE + D] = _pe_np(L, D)
    cf[:, _CF_IDN:_CF_IDN + 128] = np.eye(128, dtype=f32)
    cf[:, _CF_ONESL] = 1.0 / L
    clsw = np.asarray(inp["cls_w"], f32).reshape(D)          # (384,)
    clsw_eff = clsw * g2[NL - 1]
    cf[:, _CF_CLSW:_CF_CLSW + 3] = clsw_eff.reshape(3, 128).T
    cf[:, _CF_EPS] = EPS
    cf[0, _CF_CLSB] = float(np.asarray(inp["cls_b"], f32)[0]
                            + be2[NL - 1] @ clsw)

    # ---- per-layer blobs
    lw = np.zeros((NL, 3, 128, _LW_END), f32)
    lw[:, :, :, _LW_WQ:_LW_WQ + D] = \
        (np.asarray(inp["Wq"], f32) * TEMP).reshape(NL, 3, 128, D)
    lw[:, :, :, _LW_WK:_LW_WK + D] = np.asarray(inp["Wk"], f32).reshape(NL, 3, 128, D)
    lw[:, :, :, _LW_WV:_LW_WV + D] = np.asarray(inp["Wv"], f32).reshape(NL, 3, 128, D)
    lw[:, :, :, _LW_WO:_LW_WO + D] = Wo.reshape(NL, 3, 128, D)
    lw[:, :, :, _LW_W1:_LW_W1 + DFF] = np.asarray(inp["W1"], f32).reshape(NL, 3, 128, DFF)

    w2b = np.asarray(inp["W2"], f32).reshape(NL, 12, 128, D) \
        .transpose(0, 2, 1, 3).reshape(NL, 128, _W2_END)
    g_in = np.concatenate([np.ones((1, D), f32), g2[:NL - 1]], axis=0)   # (NL, D)
    be_in = np.concatenate([np.zeros((1, D), f32), be2[:NL - 1]], axis=0)
    sm = np.zeros((NL, 128, _SM_END), f32)
    sm[:, :, _SM_GIN:_SM_GIN + D] = np.broadcast_to(g_in[:, None, :], (NL, 128, D))
    sm[:, :, _SM_G1:_SM_G1 + D] = np.broadcast_to(g1[:, None, :], (NL, 128, D))
    bo_row = np.asarray(inp["bo"], f32) + np.einsum("ld,lde->le", bv, Wo) + be_in
    sm[:, 0, _SM_BOR:_SM_BOR + D] = bo_row
    sm[:, 0, _SM_B2R:_SM_B2R + D] = np.asarray(inp["b2"], f32) + be1

    ms = np.zeros((NL, 128, _MS_END), f32)
    bq_eff = np.asarray(inp["bq"], f32) * TEMP
    bk = np.asarray(inp["bk"], f32)
    for h in range(H):
        ms[:, :E, _MS_BQ + h] = bq_eff[:, E * h:E * h + E]
        ms[:, :E, _MS_BK + h] = bk[:, E * h:E * h + E]
    ms[:, :, _MS_GINQ:_MS_GINQ + 3] = g_in.reshape(NL, 3, 128).transpose(0, 2, 1)
    ms[:, :, _MS_BEINQ:_MS_BEINQ + 3] = be_in.reshape(NL, 3, 128).transpose(0, 2, 1)
    ms[:, :, _MS_G1Q:_MS_G1Q + 3] = g1.reshape(NL, 3, 128).transpose(0, 2, 1)
    ms[:, :, _MS_BE1Q:_MS_BE1Q + 3] = be1.reshape(NL, 3, 128).transpose(0, 2, 1)
    b1f = np.asarray(inp["b1"], f32)  # (4, 1536)
    ms[:, :, _MS_B1R:_MS_B1R + 12] = b1f.reshape(NL, 12, 128).transpose(0, 2, 1)

    return {
        "CB": cb.astype(BF),
        "CF": cf,
        "LW": lw.astype(BF),
        "W2B": w2b.astype(BF),
        "SM": sm.astype(BF),
        "MS": ms,
    }


# ---------------------------------------------------------------------------
# device program
# ---------------------------------------------------------------------------

def build_program(debug=None, do_compile=True, n_layers=NL):
    nc = bacc.Bacc("TRN2", target_bir_lowering=False, debug=False)

    x_d = nc.dram_tensor("xc", [R, W], F32, kind="ExternalInput")
    cb_d = nc.dram_tensor("CB", [128, _CB_END], BF16, kind="ExternalInput")
    cf_d = nc.dram_tensor("CF", [128, _CF_END], F32, kind="ExternalInput")
    lw_d = nc.dram_tensor("LW", [NL, 3, 128, _LW_END], BF16, kind="ExternalInput")
    w2_d = nc.dram_tensor("W2B", [NL, 128, _W2_END], BF16, kind="ExternalInput")
    sm_d = nc.dram_tensor("SM", [NL, 128, _SM_END], BF16, kind="ExternalInput")
    ms_d = nc.dram_tensor("MS", [NL, 128, _MS_END], F32, kind="ExternalInput")

    y_d = nc.dram_tensor("yc", [RPC, 1], F32, kind="ExternalOutput")
    dbg_d = None
    if debug is not None:
        dbg_d = nc.dram_tensor("dbg", [R, D], F32, kind="ExternalOutput")

    d2i = [{d: i for i, d in enumerate(conv_deltas(c))} for c in range(3)]

    from contextlib import ExitStack
    with tile.TileContext(nc) as tc, ExitStack() as ctx:
        const = ctx.enter_context(tc.tile_pool(name="const", bufs=1))
        state = ctx.enter_context(tc.tile_pool(name="state", bufs=1))
        wpool = ctx.enter_context(tc.tile_pool(name="wpool", bufs=2))
        wpool1 = ctx.enter_context(tc.tile_pool(name="wpool1", bufs=1))
        work = ctx.enter_context(tc.tile_pool(name="work", bufs=2))
        cnnw = ctx.enter_context(tc.tile_pool(name="cnnw", bufs=2))
        psum = ctx.enter_context(tc.tile_pool(name="psum", bufs=2, space="PSUM"))
        psum4 = ctx.enter_context(tc.tile_pool(name="psum4", bufs=4,
                                               space="PSUM"))

        cft = const.tile([128, _CF_END], F32, tag="cft", name="cft")
        nc.sync.dma_start(cft[:], cf_d[:])
        cbt = const.tile([128, _CB_END], BF16, tag="cbt", name="cbt")
        # split the const blob load so conv0's Toeplitz tiles land first
        nc.sync.dma_start(cbt[:, 0:_CB_T1], cb_d[:, 0:_CB_T1])
        nc.sync.dma_start(cbt[:, _CB_T1:_CB_WE], cb_d[:, _CB_T1:_CB_WE])
        nc.sync.dma_start(cbt[:, _CB_WE:_CB_END], cb_d[:, _CB_WE:_CB_END])

        def T0v(vi):
            return cbt[:, _CB_T0 + vi * 128:_CB_T0 + (vi + 1) * 128]

        def T1v(vi):
            return cbt[:, _CB_T1 + vi * 128:_CB_T1 + (vi + 1) * 128]

        def T2v(vi):
            return cbt[:, _CB_T2 + vi * 128:_CB_T2 + (vi + 1) * 128]

        Tsrc = [T0v, T1v, T2v]

        def We(c):
            return cbt[:, _CB_WE + c * D:_CB_WE + (c + 1) * D]

        idn_b = cbt[:, _CB_IDN:_CB_IDN + 128]
        ones_col = cbt[:, _CB_ONE:_CB_ONE + 1]
        ones_row = cbt[0:1, _CB_ROW1:_CB_ROW1 + 128]
        eb_row = cbt[0:1, _CB_EBR:_CB_EBR + D]
        b0e = cft[:, _CF_B0E:_CF_B0E + 1]
        b1e = cft[:, _CF_B1E:_CF_B1E + 1]
        b2e = cft[:, _CF_B2E:_CF_B2E + 1]
        pe_rm = cft[:, _CF_PE:_CF_PE + D]
        idn_f = cft[:, _CF_IDN:_CF_IDN + 128]
        onesL = cft[:, _CF_ONESL:_CF_ONESL + 1]
        clsw = cft[:, _CF_CLSW:_CF_CLSW + 3]
        epsc = cft[:, _CF_EPS:_CF_EPS + 1]
        clsb = cft[0:1, _CF_CLSB:_CF_CLSB + 1]

        # persistent state
        t_rm = [state.tile([128, D], F32, tag=f"t_rm{rt}", name=f"t_rm{rt}")
                for rt in range(RPC)]
        t_fm = [state.tile([128, R], BF16, tag=f"t_fm{c}", name=f"t_fm{c}")
                for c in range(3)]
        o_fm = [state.tile([128, R], BF16, tag=f"o_fm{c}", name=f"o_fm{c}")
                for c in range(3)]
        h1 = [state.tile([128, R], BF16, tag=f"h1_{c}", name=f"h1_{c}")
              for c in range(12)]

        # ------------------------------------------------------- CNN + embed
        # software-pipelined wavefront: stage s of row-tile rt is emitted at
        # wave rt+s so each engine's in-order stream interleaves across rts
        NBLK = (NB0, NB1, NB2)
        RELU_B = (b0e, b1e, b2e)
        cnn_st = [{} for _ in range(RPC)]  # per-rt carried tiles

        def conv_mm(conv, blk0, nblk4, src, r_all, coff):
            for g in range(nblk4):
                ps = psum.tile([128, 512], F32, tag="psA", name="psA")
                for bb in range(4):
                    b = blk0 + g * 4 + bb
                    ovl = overlaps(conv, b)
                    for i, (s, dlt) in enumerate(ovl):
                        nc.tensor.matmul(
                            ps[:, bb * 128:(bb + 1) * 128],
                            lhsT=Tsrc[conv](d2i[conv][dlt]), rhs=src(s),
                            start=(i == 0), stop=(i == len(ovl) - 1))
                nc.scalar.activation(
                    r_all[:, coff + g * 512:coff + (g + 1) * 512], ps[:],
                    AF.Relu, bias=RELU_B[conv])

        def pool_max(r_all, r_sh, dst_lo, dst_hi_ap, podd, nb2):
            # max over partition halves; evens direct, odds shifted via DMA
            nc.gpsimd.dma_start(r_sh[:], r_all[64:128, :])
            ev_lo = r_all[0:64, :].rearrange("p (b two r) -> p b two r",
                                             two=2, r=128)
            ev_sh = r_sh[:].rearrange("p (b two r) -> p b two r", two=2, r=128)
            pd_v = podd[:].rearrange("p (b r) -> p b r", b=nb2)
            nc.vector.tensor_tensor(dst_lo, ev_lo[:, :, 0, :],
                                    ev_sh[:, :, 0, :], OP.max)
            nc.vector.tensor_tensor(pd_v, ev_lo[:, :, 1, :],
                                    ev_sh[:, :, 1, :], OP.max)
            nc.gpsimd.dma_start(dst_hi_ap, podd[:])

        def cnn_stage(s, rt):
            st = cnn_st[rt]
            if s == 0:      # load + transpose
                x_t = cnnw.tile([128, W], F32, tag="x_t", name="x_t")
                nc.sync.dma_start(x_t[:], x_d[rt * 128:(rt + 1) * 128, :])
                st["xT"] = []
                for half in range(2):
                    ps = psum.tile([128, 128], F32, tag="psC", name="psC")
                    nc.tensor.transpose(
                        ps[:], x_t[:, half * 128:(half + 1) * 128], idn_f)
                    xt = cnnw.tile([128, 128], BF16, tag=f"xT{half}",
                                   name=f"xT{half}")
                    nc.vector.tensor_copy(xt[:], ps[:])
                    st["xT"].append(xt)
            elif s == 1:    # conv0
                st["rA0"] = cnnw.tile([128, NB0 * 128], BF16, tag="rA0",
                                      name="rA0")
                conv_mm(0, 0, 2, lambda i: st["xT"][i][:], st["rA0"], 0)
            elif s == 2:    # pool0
                r_sh = cnnw.tile([64, NB0 * 128], BF16, tag="rS0", name="rS0")
                pooled = cnnw.tile([128, (NB0 // 2) * 128], BF16, tag="pool0",
                                   name="pool0")
                podd = cnnw.tile([64, (NB0 // 2) * 128], BF16, tag="podd0",
                                 name="podd0")
                pool_max(st["rA0"], r_sh,
                         pooled[0:64, :].rearrange("p (b r) -> p b r", b=4),
                         pooled[64:128, :], podd, 4)
                st["pool0"] = pooled
            elif s == 3:    # conv1
                st["rA1"] = cnnw.tile([128, NB1 * 128], BF16, tag="rA1",
                                      name="rA1")
                plv = st["pool0"][:].rearrange("p (b r) -> p b r", b=4)
                conv_mm(1, 0, 4, lambda i: plv[:, i, :], st["rA1"], 0)
            elif s == 4:    # pool1
                r_sh = cnnw.tile([64, NB1 * 128], BF16, tag="rS1", name="rS1")
                pooled = cnnw.tile([128, (NB1 // 2) * 128], BF16, tag="pool1",
                                   name="pool1")
                podd = cnnw.tile([64, (NB1 // 2) * 128], BF16, tag="podd1",
                                 name="podd1")
                pool_max(st["rA1"], r_sh,
                         pooled[0:64, :].rearrange("p (b r) -> p b r", b=8),
                         pooled[64:128, :], podd, 8)
                st["pool1"] = pooled
                st["act3"] = cnnw.tile([128, 16 * 128], BF16, tag="act3",
                                       name="act3")
            elif s in (5, 6):   # conv2 half + pool into act3
                hf = s - 5
                act3 = st["act3"]
                r_all = cnnw.tile([128, 16 * 128], BF16, tag="rA2", name="rA2")
                plv = st["pool1"][:].rearrange("p (b r) -> p b r", b=8)
                conv_mm(2, hf * 16, 4, lambda i: plv[:, i, :], r_all, 0)
                r_sh = cnnw.tile([64, 16 * 128], BF16, tag="rS2", name="rS2")
                podd = cnnw.tile([64, 8 * 128], BF16, tag="a3t", name="a3t")
                pool_max(r_all, r_sh,
                         act3[0:64, hf * 1024:(hf + 1) * 1024]
                         .rearrange("p (b r) -> p b r", b=8),
                         act3[64:128, hf * 1024:(hf + 1) * 1024], podd, 8)
            elif s == 7:    # embed + relu + pe
                act3 = st["act3"]
                pse = psum4.tile([128, D], F32, tag="psB", name="psB")
                for c in range(16):
                    nc.tensor.matmul(pse[:],
                                     lhsT=act3[:, c * 128:(c + 1) * 128],
                                     rhs=We(c), start=(c == 0), stop=False)
                nc.tensor.matmul(pse[:], lhsT=ones_row, rhs=eb_row,
                                 start=False, stop=True)
                xw = work.tile([128, 2 * D], F32, tag="xw", name="xwe")
                er = xw[:, 0:D]
                nc.scalar.activation(er, pse[:], AF.Relu)
                nc.vector.tensor_tensor(t_rm[rt][:], er, pe_rm, OP.add)
                cnn_st[rt] = {}

        NSTG = 8
        for wave in range(RPC + NSTG - 1):
            for rt in range(RPC):
                s = wave - rt
                if 0 <= s < NSTG:
                    cnn_stage(s, rt)

        # ------------------------------------------------------- transformer
        for lyr in range(n_layers):
            wc = [wpool.tile([128, _LW_W1], BF16, tag=f"wc{c}", name=f"wc{c}")
                  for c in range(3)]
            w1c = [wpool1.tile([128, DFF], BF16, tag=f"w1c{c}", name=f"w1c{c}")
                   for c in range(3)]
            for c in range(3):
                nc.sync.dma_start(wc[c][:], lw_d[lyr, c, :, 0:_LW_W1])
                nc.sync.dma_start(w1c[c][:], lw_d[lyr, c, :, _LW_W1:_LW_END])
            w2t = wpool1.tile([128, _W2_END], BF16, tag="w2t", name="w2t")
            nc.sync.dma_start(w2t[:], w2_d[lyr])
            smt = wpool1.tile([128, _SM_END], BF16, tag="smt", name="smt")
            nc.sync.dma_start(smt[:], sm_d[lyr])
            ms = wpool.tile([128, _MS_END], F32, tag="ms", name="ms")
            nc.sync.dma_start(ms[:], ms_d[lyr])

            def wq(c, h):
                return wc[c][:, _LW_WQ + h * E:_LW_WQ + (h + 1) * E]

            def wk(c, h):
                return wc[c][:, _LW_WK + h * E:_LW_WK + (h + 1) * E]

            def wv(c):
                return wc[c][:, _LW_WV:_LW_WV + D]

            def wo(c):
                return wc[c][:, _LW_WO:_LW_WO + D]

            def w1(c, dc):
                return w1c[c][:, dc * 128:(dc + 1) * 128]

            def w2(dc):
                return w2t[:, dc * D:(dc + 1) * D]

            g_in_b = smt[:, _SM_GIN:_SM_GIN + D]
            g1_b = smt[:, _SM_G1:_SM_G1 + D]
            bo_row = smt[0:1, _SM_BOR:_SM_BOR + D]
            b2_row = smt[0:1, _SM_B2R:_SM_B2R + D]
            bqq = ms[:, _MS_BQ:_MS_BQ + 8]
            bkq = ms[:, _MS_BK:_MS_BK + 8]

            # t_fm <- transpose(t') with input-affine fold (g_in, be_in)
            for rt in range(RPC):
                for c in range(3):
                    ps = psum.tile([128, 128], F32, tag="psC", name="psC")
                    nc.tensor.transpose(ps[:], t_rm[rt][:, c * 128:(c + 1) * 128],
                                        idn_f)
                    nc.scalar.activation(
                        t_fm[c][:, rt * 128:(rt + 1) * 128], ps[:], AF.Identity,
                        scale=ms[:, _MS_GINQ + c:_MS_GINQ + c + 1],
                        bias=ms[:, _MS_BEINQ + c:_MS_BEINQ + c + 1])

            # attention, two token-block passes (samples 4tb..4tb+3)
            for tb in range(2):
                ts = slice(tb * 512, (tb + 1) * 512)
                # Q^T / K^T feature-major for this token block (M=48 lhsT)
                qf = state.tile([48, H * 512], BF16, tag="qf", name="qf")
                kf = state.tile([48, H * 512], BF16, tag="kf", name="kf")
                for dst, wmat, bias in ((qf, wq, bqq), (kf, wk, bkq)):
                    for h in range(H):
                        pq = psum.tile([48, 512], F32, tag="psA", name="psQ")
                        for c in range(3):
                            nc.tensor.matmul(
                                pq[:], lhsT=wmat(c, h), rhs=t_fm[c][:, ts],
                                start=(c == 0), stop=(c == 2))
                        nc.scalar.activation(
                            dst[:, h * 512:(h + 1) * 512], pq[:], AF.Identity,
                            bias=bias[0:48, h:h + 1])

                for nn in range(4):
                    n = tb * 4 + nn
                    cs = slice(n * 128, (n + 1) * 128)
                    qs = slice(nn * 128, (nn + 1) * 128)
                    # V row-major (bv folded into bo_row via Wo)
                    pv = psum4.tile([128, D], F32, tag="psB", name="psB")
                    for c in range(3):
                        nc.tensor.matmul(pv[:], lhsT=t_fm[c][:, cs], rhs=wv(c),
                                         start=(c == 0), stop=(c == 2))
                    # v_aug: per-head [48 V cols | 1 ones col] so the AV
                    # matmul emits softmax denominators in the same pass
                    v_rm = work.tile([128, H * 49], BF16, tag="v_rm",
                                     name="v_rm")
                    nc.vector.tensor_copy(
                        v_rm[:].rearrange("p (h e) -> p h e", h=H)[:, :, 0:E],
                        pv[:].rearrange("p (h e) -> p h e", h=H))
                    nc.vector.memset(
                        v_rm[:].rearrange("p (h e) -> p h e", h=H)[:, :, E:49],
                        1.0)

                    # S^T per head (partitions = key idx) then exp -> es16
                    es16 = work.tile([128, H * 128], BF16, tag="es16",
                                     name="es16")
                    for half in range(2):
                        pss = psum.tile([128, 512], F32, tag="psA", name="psA")
                        for hh in range(4):
                            h = half * 4 + hh
                            nc.tensor.matmul(
                                pss[:, hh * 128:(hh + 1) * 128],
                                lhsT=kf[:, h * 512 + nn * 128:
                                        h * 512 + (nn + 1) * 128],
                                rhs=qf[:, h * 512 + nn * 128:
                                       h * 512 + (nn + 1) * 128],
                                start=True, stop=True)
                        nc.scalar.activation(
                            es16[:, half * 512:(half + 1) * 512], pss[:],
                            AF.Exp)

                    # AV: [o_h | rowsum_h] per head in one psum bank
                    pso = psum4.tile([128, H * 49], F32, tag="psB", name="psO")
                    for h in range(H):
                        nc.tensor.matmul(pso[:, h * 49:(h + 1) * 49],
                                         lhsT=es16[:, h * 128:(h + 1) * 128],
                                         rhs=v_rm[:, h * 49:(h + 1) * 49],
                                         start=True, stop=True)
                    sc = work.tile([128, 20], F32, tag="sc", name="sc")
                    rr = sc[:, 10:18]
                    psv = pso[:].rearrange("p (h e) -> p h e", h=H)
                    nc.vector.reciprocal(
                        rr, psv[:, :, E:49].rearrange("p a b -> p (a b)"))
                    o_rm = work.tile([128, D], BF16, tag="o_rm", name="o_rm")
                    rrb = AP(rr.tensor, rr.offset,
                             [list(rr.ap[0]), [1, 8], [0, E]])
                    nc.vector.tensor_tensor(
                        o_rm[:].rearrange("p (a b) -> p a b", a=8),
                        psv[:, :, 0:E], rrb, OP.mult)
                    for c in range(3):
                        ps = psum.tile([128, 128], BF16, tag="psC", name="psC")
                        nc.tensor.transpose(ps[:],
                                            o_rm[:, c * 128:(c + 1) * 128],
                                            idn_b)
                        nc.vector.tensor_copy(o_fm[c][:, cs], ps[:])

            # u = o @ Wo + bo_row ; x1 = t_eff + u ; LN1 -> t_rm (=t1')
            def layer_norm(rt, x1):
                sc = work.tile([128, 20], F32, tag="sc", name="scln")
                bnt, ag = sc[:, 0:6], sc[:, 6:8]
                sd, rstd = sc[:, 8:9], sc[:, 9:10]
                nc.vector.bn_stats(bnt, x1)
                nc.vector.bn_aggr(ag, bnt)
                nc.scalar.activation(sd, ag[:, 1:2], AF.Sqrt, bias=epsc)
                nc.vector.reciprocal(rstd, sd)
                nc.vector.tensor_scalar(t_rm[rt][:], x1, ag[:, 0:1], rstd,
                                        OP.subtract, OP.mult)

            for rt in range(RPC):
                cs = slice(rt * 128, (rt + 1) * 128)
                pu = psum4.tile([128, D], F32, tag="psB", name="psB")
                for c in range(3):
                    nc.tensor.matmul(pu[:], lhsT=o_fm[c][:, cs], rhs=wo(c),
                                     start=(c == 0), stop=False)
                nc.tensor.matmul(pu[:], lhsT=ones_row, rhs=bo_row,
                                 start=False, stop=True)
                xw = work.tile([128, 2 * D], F32, tag="xw", name="xw")
                x1 = xw[:, 0:D]
                if lyr == 0:
                    nc.vector.tensor_tensor(x1, pu[:], t_rm[rt][:], OP.add)
                else:
                    tmp = xw[:, D:2 * D]
                    nc.vector.tensor_tensor(tmp, t_rm[rt][:], g_in_b, OP.mult)
                    nc.vector.tensor_tensor(x1, pu[:], tmp, OP.add)
                layer_norm(rt, x1)

            # FFN: t_fm <- transpose(t1') with (g1, be1) fold
            for rt in range(RPC):
                for c in range(3):
                    ps = psum.tile([128, 128], F32, tag="psC", name="psC")
                    nc.tensor.transpose(ps[:], t_rm[rt][:, c * 128:(c + 1) * 128],
                                        idn_f)
                    nc.scalar.activation(
                        t_fm[c][:, rt * 128:(rt + 1) * 128], ps[:], AF.Identity,
                        scale=ms[:, _MS_G1Q + c:_MS_G1Q + c + 1],
                        bias=ms[:, _MS_BE1Q + c:_MS_BE1Q + c + 1])
            for dc in range(12):
                ph = [psum.tile([128, 512], F32, tag="psA", name="psH")
                      for _ in range(2)]
                for c in range(3):
                    for b in range(2):
                        nc.tensor.matmul(ph[b][:], lhsT=w1(c, dc),
                                         rhs=t_fm[c][:, b * 512:(b + 1) * 512],
                                         start=(c == 0), stop=(c == 2))
                for b in range(2):
                    nc.scalar.activation(h1[dc][:, b * 512:(b + 1) * 512],
                                         ph[b][:], AF.Relu,
                                         bias=ms[:, _MS_B1R + dc:_MS_B1R + dc + 1])
            for rt in range(RPC):
                cs = slice(rt * 128, (rt + 1) * 128)
                py = psum4.tile([128, D], F32, tag="psB", name="psB")
                for dc in range(12):
                    nc.tensor.matmul(py[:], lhsT=h1[dc][:, cs], rhs=w2(dc),
                                     start=(dc == 0), stop=False)
                nc.tensor.matmul(py[:], lhsT=ones_row, rhs=b2_row,
                                 start=False, stop=True)
                xw = work.tile([128, 2 * D], F32, tag="xw", name="xw")
                x2, tmp = xw[:, 0:D], xw[:, D:2 * D]
                nc.vector.tensor_tensor(tmp, t_rm[rt][:], g1_b, OP.mult)
                nc.vector.tensor_tensor(x2, py[:], tmp, OP.add)
                layer_norm(rt, x2)

        if dbg_d is not None:
            for rt in range(RPC):
                nc.sync.dma_start(dbg_d[rt * 128:(rt + 1) * 128, :], t_rm[rt][:])

        # ------------------------------------------------------- head
        # (cls weights pre-folded with last-layer LN affine)
        outsb = state.tile([1, RPC], F32, tag="outsb", name="outsb")
        for n in range(RPC):
            pm = psum.tile([128, 3], F32, tag="psC", name="psC2")
            for c in range(3):
                nc.tensor.matmul(pm[:, c:c + 1],
                                 lhsT=t_rm[n][:, c * 128:(c + 1) * 128],
                                 rhs=onesL, start=True, stop=True)
            sc = work.tile([128, 20], F32, tag="sc", name="schd")
            tm = sc[:, 0:3]
            nc.scalar.copy(tm, pm[:])
            pc = psum.tile([1, 8], F32, tag="psC", name="psC3")
            for c in range(3):
                nc.tensor.matmul(pc[:, 0:1], lhsT=tm[:, c:c + 1],
                                 rhs=clsw[:, c:c + 1],
                                 start=(c == 0), stop=(c == 2))
            nc.scalar.activation(outsb[:, n:n + 1], pc[:, 0:1], AF.Identity,
                                 bias=clsb)
        nc.sync.dma_start(y_d[:].rearrange("a b -> b a"), outsb[:])

    if do_compile:
        nc.compile()
    return nc


_PROG = {}


def _get_prog(debug=None, n_layers=NL):
    key = ("dbg" if debug else "plain", n_layers)
    if key not in _PROG:
        _PROG[key] = build_program(debug, n_layers=n_layers)
    return _PROG[key]


def _in_maps(inputs):
    shared = host_prep(inputs)
    x = np.asarray(inputs["x"], np.float32)  # (64, 128, 256)
    in_maps = []
    for c in range(NCORES):
        m = dict(shared)
        m["xc"] = np.ascontiguousarray(
            x[c * RPC:(c + 1) * RPC].reshape(R, W))
        in_maps.append(m)
    return in_maps


def kernel(**inputs):
    nc = _get_prog()
    res = run_bass_kernel_spmd(nc, _in_maps(inputs), core_ids=list(range(NCORES)))
    out = np.concatenate([res.results[c]["yc"] for c in range(NCORES)], axis=0)
    return out.astype(np.float32)


def timed_run(inputs, iters=12):
    """Wall-clock the sharded PJRT dispatch with device-resident inputs.

    No NTFF hook is available through this axon tunnel, so this measures
    dispatch+execute wall time; min over iters approximates HW exec + fixed
    dispatch overhead.  Returns ns.
    """
    import time
    import jax
    import jax.numpy as jnp
    from jax.experimental.shard_map import shard_map
    from jax.sharding import Mesh, NamedSharding, PartitionSpec
    from concourse import bass2jax, mybir as mb

    nc = _get_prog()
    bass2jax.install_neuronx_cc_hook()
    in_maps = _in_maps(inputs)
    partition_name = nc.partition_id_tensor.name if nc.partition_id_tensor else None
    in_names, out_names, out_avals, zero_outs = [], [], [], []
    for alloc in nc.m.functions[0].allocations:
        if not isinstance(alloc, mb.MemoryLocationSet):
            continue
        name = alloc.memorylocations[0].name
        if alloc.kind == "ExternalInput":
            if name != partition_name:
                in_names.append(name)
        elif alloc.kind == "ExternalOutput":
            shape = tuple(alloc.tensor_shape)
            dtype = mb.dt.np(alloc.dtype)
            out_avals.append(jax.core.ShapedArray(shape, dtype))
            out_names.append(name)
            zero_outs.append(np.zeros(shape, dtype))
    n_params, n_outs = len(in_names), len(out_avals)
    all_in = list(in_names) + list(out_names)
    if partition_name is not None:
        all_in.append(partition_name)

    def _body(*args):
        ins = list(args[:n_params])
        outs = list(args[n_params:])
        operands = ins + outs
        if partition_name is not None:
            operands = operands + [bass2jax.partition_id_tensor()]
        outs = list(bass2jax._bass_exec_p.bind(
            *operands, out_avals=tuple(out_avals), in_names=tuple(all_in),
            out_names=tuple(out_names), lowering_input_output_aliases=(),
            sim_require_finite=True, sim_require_nnan=True, nc=nc))
        return tuple(outs)

    devices = jax.devices()[:NCORES]
    mesh = Mesh(np.asarray(devices), ("core",))
    shard = NamedSharding(mesh, PartitionSpec("core"))
    dev_in = [jax.device_put(
        np.concatenate([np.asarray(in_maps[c][nm]) for c in range(NCORES)], axis=0),
        shard) for nm in in_names]
    zsh = [np.zeros((NCORES * z.shape[0], *z.shape[1:]), z.dtype) for z in zero_outs]
    f = jax.jit(
        shard_map(_body, mesh=mesh,
                  in_specs=(PartitionSpec("core"),) * (n_params + n_outs),
                  out_specs=(PartitionSpec("core"),) * n_outs, check_rep=False),
        keep_unused=True)
    zs = [jax.device_put(z, shard) for z in zsh]
    out = f(*dev_in, *zs)
    jax.block_until_ready(out)
    ts = []
    for _ in range(iters):
        t0 = time.perf_counter()
        out = f(*dev_in, *zs)
        jax.block_until_ready(out)
        ts.append(time.perf_counter() - t0)
    return int(min(ts) * 1e9)


def debug_run(inputs, core=0, n_layers=NL, ncores=1):
    """Run the debug program; returns (y, t_rm_dump) for one core."""
    nc = _get_prog(debug=True, n_layers=n_layers)
    res = run_bass_kernel_spmd(nc, _in_maps(inputs)[:ncores],
                               core_ids=list(range(ncores)))
    return res.results[core]["yc"], res.results[core]["dbg"]
